# revision 25
# baseline (speedup 1.0000x reference)
"""Causal self-attention (B=2, T=2048, C=1024, NH=16, HD=64) on 8 NeuronCores.

Sharding: core c -> (batch b = c//4, head-group hg = c%4 of 4 heads).
Each core computes qkv projection for its 4 heads from x[b], attention for
its 4 (b,h) units, and a partial output projection (row-parallel over the
head dim). Unshard = sum of the 4 partials per batch (bproj/4 folded in).

Per-core device algorithm (v3, bf16 data path):
  Host pre-casts x, Wqkv, Wproj shards to bf16 (rel-err budget ~4e-3 vs
  2e-2 gate; halves input DMA bytes).
  A. x[b] transposed on PE (128x128 bf16 tiles, 1 cyc/row) -> xT in SBUF
     (bf16 PSUM -> 2x-mode DVE evict).
  B. qkT [512, 2048] = wqk.T @ xT; DVE evict adds bias, keeps bf16.
  C. v_aug [2048, 260] = [x[b] | 1] @ wv_aug; ACT evict -> bf16 (per head
     64 v cols + a ones col: softmax sums as a matmul byproduct; v bias
     folded into the aug row).
  D. Attention in i-chunks (3x512 then 256/128/128 so the tail exposes
     little work): att^T[j,i] blocks via PE (K=64), j-tiles processed in
     pairs sharing a 2-bank PSUM so exp runs once per pair on ACT (scale
     1/8, no max subtraction -- logits are O(1) by construction) -> e bf16,
     causal mask = multiplicative bf16 mask on DVE (2x mode), y^T
     accumulation [65, w] (row 64 = softmax sums S).  Normalize: DVE
     recip(S) -> Pool partition_broadcast -> DVE mul -> yT bf16.
  E. out_partial = yT.T @ wp + bproj/4 per token-tile as soon as its chunk
     is normalized; f32 evict+bias on DVE, store via SP HWDGE DMA.
  Emission zips ABC units of quarter Q with attention units of the prior
  chunk so the Tile scheduler always has PE filler during exp stalls.
"""
import os
import sys

import numpy as np

for _p in ("/opt/trn_rl_repo",):
    if _p not in sys.path and os.path.isdir(_p):
        sys.path.insert(0, _p)

import concourse.bass as bass
import concourse.mybir as mybir
import concourse.tile as tile
from concourse.masks import make_identity

B, T, C, NH, HD = 2, 2048, 1024, 16, 64
F32 = mybir.dt.float32
BF16 = mybir.dt.bfloat16
N_CORES = 8
NT = T // 128   # 16 token tiles
NQ = 4          # token quarters (512 tokens each)
NKT = C // 128  # 8 contraction tiles
EXP_SCALE = float(HD) ** -0.5
EXP = mybir.ActivationFunctionType.Exp
# attention i-chunks (start token-tile, tile count): taper the tail
CHUNKS = [(0, 4), (4, 4), (8, 4), (12, 2), (14, 1), (15, 1)]


def build_nc(split_waits=True):
    nc = bass.Bass()
    xb_d = nc.declare_dram_parameter("xb", [T, C], BF16, isOutput=False)
    wqk_d = nc.declare_dram_parameter("wqk", [C, 512], BF16, isOutput=False)
    bqk_d = nc.declare_dram_parameter("bqk", [512], F32, isOutput=False)
    wv_d = nc.declare_dram_parameter("wv", [C, 256], BF16, isOutput=False)
    wp_d = nc.declare_dram_parameter("wp", [256, C], BF16, isOutput=False)
    bp4_d = nc.declare_dram_parameter("bp4", [1, C], F32, isOutput=False)
    ident_d = nc.declare_dram_parameter("ident", [128, 128], BF16, isOutput=False)
    maskf_d = nc.declare_dram_parameter("maskf", [128, 896], BF16, isOutput=False)
    out_d = nc.declare_dram_parameter("out", [T, C], F32, isOutput=True)

    with tile.TileContext(nc) as tc:
        with (
            tc.tile_pool(name="const", bufs=1) as const,
            tc.tile_pool(name="wts", bufs=1) as wts,
            tc.tile_pool(name="xbs", bufs=1) as xbs,
            tc.tile_pool(name="xbp", bufs=3) as xbp,
            tc.tile_pool(name="xtp", bufs=2) as xtp,
            tc.tile_pool(name="qkt", bufs=1) as qkt,
            tc.tile_pool(name="vsb", bufs=1) as vsb,
            tc.tile_pool(name="ep", bufs=8) as ep,
            tc.tile_pool(name="rsp", bufs=3) as rsp,
            tc.tile_pool(name="ytp", bufs=1) as ytp,
            tc.tile_pool(name="outp", bufs=4) as outp,
            tc.tile_pool(name="psA", bufs=2, space="PSUM") as psA,
            tc.tile_pool(name="psE", bufs=2, space="PSUM") as psE,
            tc.tile_pool(name="psY", bufs=2, space="PSUM") as psY,
        ):
            # ---- x0 + ident first (gate the first transposes) ----
            xb_tiles = [None] * NT
            ident = const.tile([128, 128], BF16, name="ident")
            def load_x_tile(t):
                tl = xbs.tile([128, C], BF16, name="xb_t", tag=f"xbs{t}")
                nc.sync.dma_start(out=tl[:], in_=xb_d[t * 128:(t + 1) * 128, :])
                xb_tiles[t] = (tl, 0)
            def load_x_pair(t):
                tl = xbp.tile([128, 2 * C], BF16, name="xb_p", tag="xb_p")
                nc.sync.dma_start(
                    out=tl[:],
                    in_=xb_d[t * 128:(t + 2) * 128, :].rearrange(
                        "(a p) c -> p a c", p=128))
                xb_tiles[t] = (tl, 0)
                xb_tiles[t + 1] = (tl, 1)
            load_x_tile(0)
            nc.sync.dma_start(out=ident[:], in_=ident_d[:, :])
            for t in range(1, 4):
                load_x_tile(t)

            # ---- weights (ordered by first use) ----
            wqk_sb = []
            for h in range(2):
                w = wts.tile([128, 2048], BF16, name=f"wqk{h}", tag=f"wqk{h}")
                nc.sync.dma_start(
                    out=w[:],
                    in_=wqk_d[h * 512:(h + 1) * 512, :].rearrange(
                        "(a p) n -> p a n", p=128))
                wqk_sb.append(w)
            def wqk_ap(kt, p0, p1):
                return wqk_sb[kt // 4][:, (kt % 4) * 512 + p0:(kt % 4) * 512 + p1]

            bqk_sb = const.tile([128, 4], F32, name="bqk_sb")
            nc.sync.dma_start(out=bqk_sb[:], in_=bqk_d.rearrange("(t p) -> p t", p=128))

            wv_sb = []
            for h in range(2):
                w = wts.tile([128, 4 * 256], BF16, name=f"wv{h}", tag=f"wv{h}")
                nc.sync.dma_start(
                    out=w[:],
                    in_=wv_d[h * 512:(h + 1) * 512, :].rearrange(
                        "(a p) n -> p a n", p=128))
                wv_sb.append(w)
            def wv_ap(kt):
                return wv_sb[kt // 4][:, (kt % 4) * 256:(kt % 4 + 1) * 256]

            wp_sb = wts.tile([128, 2048], BF16, name="wp", tag="wp")
            nc.sync.dma_start(
                out=wp_sb[:],
                in_=wp_d[:, :].rearrange("(a p) n -> p a n", p=128))
            def wp_ap(kt, n0, n1):
                return wp_sb[:, kt * C + n0:kt * C + n1]

            bp_row = const.tile([1, C], F32, name="bp_row")
            nc.sync.dma_start(out=bp_row[:], in_=bp4_d[:, :])

            # remaining x tiles
            for t in range(4, NT, 2):
                load_x_pair(t)

            # ---- constants (ident/mask DMA'd: keeps Pool off the
            # startup critical path) ----
            maskf = const.tile([128, 896], BF16, name="maskf")
            nc.sync.dma_start(out=maskf[:], in_=maskf_d[:, :])
            # proj bias broadcast tile [128, C] via Pool SWDGE repeat-DMA
            bp_sb = const.tile([128, C], F32, name="bp_sb")
            nc.gpsimd.dma_start(
                out=bp_sb[:],
                in_=bp_row[0:1, :].unsqueeze(1).broadcast_to([1, 128, C]))

            # ---- persistent activations ----
            qkT = [qkt.tile([128, T], BF16, name=f"qkT{p}", tag=f"qkT{p}")
                   for p in range(4)]
            v_sb = [vsb.tile([128, 512], BF16, name=f"v{jt}", tag=f"v{jt}")
                    for jt in range(NT)]
            for jt in range(NT):
                # ones columns 64..127 of each head block: the y matmul then
                # replicates the softmax sums into out partitions 64..127
                nc.gpsimd.memset(
                    v_sb[jt][:].rearrange("p (s c) -> p s c", s=4)[:, :, 64:128],
                    1.0)
            yT = [[ytp.tile([128, 512], BF16, name=f"yT{ic}_{kt}",
                            tag=f"yT{ic}_{kt}") for kt in range(2)]
                  for ic in range(NQ)]

            # ---- attention for i-chunk [t0*128, (t0+nt)*128) as units ----
            def attn_chunk_units(t0, nt, split=False):
                w = nt * 128
                i0 = t0 * 128
                qq = t0 // 4
                units = []
                for s in range(4):
                    qrow = (s % 2) * 64
                    qtile = qkT[s // 2]
                    ktile = qkT[2 + s // 2]
                    n_jt = t0 + nt
                    state = {}

                    gsz = max(2, 1024 // w)

                    def pair_unit(g0, s=s, qrow=qrow, qtile=qtile,
                                  ktile=ktile, n_jt=n_jt, state=state,
                                  gsz=gsz):
                        if g0 == 0:
                            state["ps_y"] = psY.tile([128, 512], F32,
                                                     name="ps_y", tag="psY")
                            state["n_y"] = 0
                        ps_y = state["ps_y"]
                        pair = list(range(g0, min(g0 + gsz, n_jt)))
                        ps_a = psE.tile([128, 1024], F32, name="ps_a",
                                        tag="psE")
                        e = ep.tile([128, 1024], BF16, name="e_t", tag="e_t")
                        blocks = []
                        for h, jt in enumerate(pair):
                            o = max(0, (jt - t0) * 128)
                            nc.tensor.matmul(
                                ps_a[:, h * w + o:(h + 1) * w],
                                ktile[qrow:qrow + 64,
                                      jt * 128:(jt + 1) * 128],
                                qtile[qrow:qrow + 64, i0 + o:i0 + w],
                                start=True, stop=True,
                            )
                            blocks.append((jt, h, o))
                        if all(o == 0 for _, _, o in blocks):
                            nc.scalar.activation(
                                e[:, 0:len(pair) * w],
                                ps_a[:, 0:len(pair) * w],
                                EXP, scale=EXP_SCALE)
                        else:
                            for jt, h, o in blocks:
                                nc.scalar.activation(
                                    e[:, h * w + o:(h + 1) * w],
                                    ps_a[:, h * w + o:(h + 1) * w],
                                    EXP, scale=EXP_SCALE)
                        for jt, h, o in blocks:
                            if jt >= t0:
                                nc.vector.tensor_mul(
                                    e[:, h * w + o:(h + 1) * w],
                                    e[:, h * w + o:(h + 1) * w],
                                    maskf[:, 384:384 + w - o])
                            nc.tensor.matmul(
                                ps_y[:, o:w],
                                v_sb[jt][:, s * 128:s * 128 + 128],
                                e[:, h * w + o:(h + 1) * w],
                                start=(state["n_y"] == 0),
                                stop=(state["n_y"] == n_jt - 1),
                            )
                            state["n_y"] += 1

                    for g0 in range(0, n_jt, gsz):
                        units.append(lambda g0=g0, f=pair_unit: f(g0))

                    def norm_unit(s=s, qrow=qrow, state=state):
                        ps_y = state["ps_y"]
                        r_sb = rsp.tile([64, 512], BF16, name="r_sb",
                                        tag="r_sb")
                        with nc.allow_low_precision(
                                reason="softmax sum recip is well-conditioned"):
                            nc.vector.reciprocal(r_sb[:, 0:w],
                                                 ps_y[64:128, 0:w])
                        nc.vector.tensor_mul(
                            yT[qq][s // 2][qrow:qrow + 64,
                                           (t0 % 4) * 128:(t0 % 4) * 128 + w],
                            ps_y[0:64, 0:w], r_sb[:, 0:w])
                    units.append(norm_unit)

                # projection for this chunk's token tiles
                for mt in range(t0, t0 + nt):
                    def proj_unit(mt=mt, qq=qq):
                        mtl = mt % 4
                        o = outp.tile([128, C], F32, name="o_t", tag="o_t")
                        for nch in range(2):
                            sl = slice(nch * 512, (nch + 1) * 512)
                            ps = psA.tile([128, 512], F32, name="ps_o",
                                          tag="psA")
                            for kt in range(2):
                                nc.tensor.matmul(
                                    ps[:],
                                    yT[qq][kt][:, mtl * 128:(mtl + 1) * 128],
                                    wp_ap(kt, nch * 512, (nch + 1) * 512),
                                    start=(kt == 0), stop=(kt == 1),
                                )
                            nc.vector.tensor_add(o[:, sl], ps[:], bp_sb[:, sl])
                            nc.sync.dma_start(
                                out=out_d[mt * 128:(mt + 1) * 128, sl],
                                in_=o[:, sl])
                    units.append(proj_unit)
                if split:
                    return units[:-nt], units[-nt:]
                return units

            # ---- ABC units for one token-quarter ----
            def abc_units(Q):
                units = []
                xT_q = []

                def a_unit(kt, Q=Q, xT_q=xT_q):
                    xt = xtp.tile([128, 512], BF16, name="xT_t", tag=f"xT{kt}")
                    ps = psA.tile([128, 512], BF16, name="ps_tr", tag="psA")
                    for q in range(4):
                        tl, a = xb_tiles[4 * Q + q]
                        nc.tensor.transpose(
                            ps[:, q * 128:(q + 1) * 128],
                            tl[:, a * C + kt * 128:a * C + (kt + 1) * 128],
                            ident[:],
                        )
                    nc.vector.tensor_copy(xt[:], ps[:])
                    xT_q.append(xt)

                def b_unit(p, Q=Q, xT_q=xT_q):
                    ps = psA.tile([128, 512], F32, name="ps_qk", tag="psA")
                    for kt in range(NKT):
                        nc.tensor.matmul(
                            ps[:], wqk_ap(kt, p * 128, (p + 1) * 128),
                            xT_q[kt][:],
                            start=(kt == 0), stop=(kt == NKT - 1),
                        )
                    nc.scalar.activation(
                        qkT[p][:, Q * 512:(Q + 1) * 512], ps[:],
                        mybir.ActivationFunctionType.Identity,
                        bias=bqk_sb[:, p:p + 1])

                def c_unit(mt, Q=Q, xT_q=xT_q):
                    jt = 4 * Q + mt
                    ps = psA.tile([128, 256], F32, name="ps_v", tag="psA")
                    for kt in range(NKT):
                        nc.tensor.matmul(
                            ps[:], xT_q[kt][:, mt * 128:(mt + 1) * 128],
                            wv_ap(kt),
                            start=(kt == 0), stop=(kt == NKT - 1),
                        )
                    nc.scalar.copy(
                        v_sb[jt][:].rearrange("p (s c) -> p s c",
                                              s=4)[:, :, 0:64],
                        ps[:].rearrange("p (s c) -> p s c", s=4))

                for kt in range(NKT):
                    units.append(lambda kt=kt: a_unit(kt))
                for p in range(4):
                    units.append(lambda p=p: b_unit(p))
                for mt in range(4):
                    units.append(lambda mt=mt: c_unit(mt))
                return units

            def zip_emit(ua, ub):
                na, nb = len(ua), len(ub)
                ia = ib = 0
                while ia < na or ib < nb:
                    if ib >= nb or (ia < na and ia * nb <= ib * na):
                        ua[ia]()
                        ia += 1
                    else:
                        ub[ib]()
                        ib += 1

            # ---- main schedule: zip quarter ABC with prior chunk's attn;
            # chunk (8,4)'s projection is deferred into the tail as PE
            # filler for the ACT-bound final chunks ----
            for u in abc_units(0):
                u()
            zip_emit(abc_units(1), attn_chunk_units(0, 4))
            zip_emit(abc_units(2), attn_chunk_units(4, 4))
            au, pu = attn_chunk_units(8, 4, split=True)
            zip_emit(abc_units(3), au)
            tail = []
            for t0, nt in CHUNKS[3:]:
                tail.extend(attn_chunk_units(t0, nt))
            zip_emit(tail, pu)

    if split_waits:
        _split_matmul_waits(nc)
    return nc


def _split_matmul_waits(nc):
    """Walrus codegen in this pipeline allows only one sync wait per
    instruction for most ISA structs (S3_LW, PSEUDO_DMA_DIRECT2D, S3D3_TS,
    ...). Move extra waits onto inserted NoOps on the same engine (program
    order preserves semantics)."""
    n_split = 0
    for bb in nc.main_func.blocks:
        out = []
        for ins in bb.instructions:
            si = getattr(ins, "sync_info", None)
            if (si is not None and len(si.on_wait) >= 2
                    and type(ins).__name__ != "InstNoOp"):
                for w in si.on_wait[:-1]:
                    nop = mybir.InstNoOp(name=f"I-wsplit-{nc.next_id()}",
                                         ins=[], outs=[])
                    nop.engine = ins.engine
                    nop.sync_info = mybir.SyncInfo(on_wait=[w], on_update=[])
                    out.append(nop)
                    n_split += 1
                ins.sync_info = mybir.SyncInfo(
                    on_wait=[si.on_wait[-1]], on_update=si.on_update)
            out.append(ins)
        bb.instructions[:] = out
    return n_split


def shard_inputs(x, Wqkv, bqkv, Wproj, bproj):
    import ml_dtypes
    BF = ml_dtypes.bfloat16
    x = np.ascontiguousarray(np.asarray(x, np.float32))
    Wqkv = np.asarray(Wqkv, np.float32)
    bqkv = np.asarray(bqkv, np.float32)
    Wproj = np.asarray(Wproj, np.float32)
    bproj = np.asarray(bproj, np.float32)
    in_maps = []
    for c in range(N_CORES):
        b, hg = c // 4, c % 4
        wqk = np.ascontiguousarray(np.concatenate(
            [Wqkv[:, hg * 256:(hg + 1) * 256],
             Wqkv[:, C + hg * 256: C + (hg + 1) * 256]], axis=1).astype(BF))
        bqk = np.ascontiguousarray(np.concatenate(
            [bqkv[hg * 256:(hg + 1) * 256],
             bqkv[C + hg * 256: C + (hg + 1) * 256]]))
        wv = np.zeros((C, 256), np.float32)
        bv = np.zeros(256, np.float32)
        for s in range(4):
            h = 4 * hg + s
            wv[:, s * 64:s * 64 + 64] = Wqkv[:, 2 * C + h * 64: 2 * C + (h + 1) * 64]
            bv[s * 64:s * 64 + 64] = bqkv[2 * C + h * 64: 2 * C + (h + 1) * 64]
        wp = np.ascontiguousarray(Wproj[hg * 256:(hg + 1) * 256, :].astype(BF))
        # v-bias enters y additively (softmax rows sum to 1): fold exactly
        # into this core's projection bias partial.
        bp4 = np.ascontiguousarray(
            (bproj / 4 + bv @ Wproj[hg * 256:(hg + 1) * 256, :])[None, :])
        ident = np.ascontiguousarray(np.eye(128, dtype=np.float32).astype(BF))
        maskf = np.zeros((128, 896), np.float32)
        for j in range(128):
            maskf[j, j + 384:] = 1.0
        in_maps.append({"xb": np.ascontiguousarray(x[b].astype(BF)),
                        "wqk": wqk, "bqk": bqk,
                        "wv": np.ascontiguousarray(wv.astype(BF)),
                        "wp": wp, "bp4": bp4,
                        "ident": ident,
                        "maskf": np.ascontiguousarray(maskf.astype(BF))})
    return in_maps


_NC_CACHE = {}


def kernel(x, Wqkv, bqkv, Wproj, bproj):
    from concourse.bass_utils import run_bass_kernel_spmd

    if "nc" not in _NC_CACHE:
        _NC_CACHE["nc"] = build_nc()
    nc = _NC_CACHE["nc"]
    in_maps = shard_inputs(x, Wqkv, bqkv, Wproj, bproj)
    res = run_bass_kernel_spmd(nc, in_maps, list(range(N_CORES)))
    _NC_CACHE["last_exec_time_ns"] = res.exec_time_ns
    out = np.zeros((B, T, C), np.float32)
    for c in range(N_CORES):
        out[c // 4] += res.results[c]["out"]
    return out


# revision 32
# speedup vs baseline: 1.0485x; 1.0485x over previous
"""Causal self-attention (B=2, T=2048, C=1024, NH=16, HD=64) on 8 NeuronCores.

Sharding: core c -> (batch b = c//4, head-group hg = c%4 of 4 heads).
Each core computes qkv projection for its 4 heads from x[b], attention for
its 4 (b,h) units, and a partial output projection (row-parallel over the
head dim). Unshard = sum of the 4 partials per batch (bproj/4 folded in).

Per-core device algorithm (v3, bf16 data path):
  Host pre-casts x, Wqkv, Wproj shards to bf16 (rel-err budget ~4e-3 vs
  2e-2 gate; halves input DMA bytes).
  A. x[b] transposed on PE (128x128 bf16 tiles, 1 cyc/row) -> xT in SBUF
     (bf16 PSUM -> 2x-mode DVE evict).
  B. qkT [512, 2048] = wqk.T @ xT; DVE evict adds bias, keeps bf16.
  C. v_aug [2048, 260] = [x[b] | 1] @ wv_aug; ACT evict -> bf16 (per head
     64 v cols + a ones col: softmax sums as a matmul byproduct; v bias
     folded into the aug row).
  D. Attention in i-chunks (3x512 then 256/128/128 so the tail exposes
     little work): att^T[j,i] blocks via PE (K=64), j-tiles processed in
     pairs sharing a 2-bank PSUM so exp runs once per pair on ACT (scale
     1/8, no max subtraction -- logits are O(1) by construction) -> e bf16,
     causal mask = multiplicative bf16 mask on DVE (2x mode), y^T
     accumulation [65, w] (row 64 = softmax sums S).  Normalize: DVE
     recip(S) -> Pool partition_broadcast -> DVE mul -> yT bf16.
  E. out_partial = yT.T @ wp + bproj/4 per token-tile as soon as its chunk
     is normalized; f32 evict+bias on DVE, store via SP HWDGE DMA.
  Emission zips ABC units of quarter Q with attention units of the prior
  chunk so the Tile scheduler always has PE filler during exp stalls.
"""
import os
import sys

import numpy as np

for _p in ("/opt/trn_rl_repo",):
    if _p not in sys.path and os.path.isdir(_p):
        sys.path.insert(0, _p)

import concourse.bass as bass
import concourse.mybir as mybir
import concourse.tile as tile
from concourse.masks import make_identity

B, T, C, NH, HD = 2, 2048, 1024, 16, 64
F32 = mybir.dt.float32
BF16 = mybir.dt.bfloat16
N_CORES = 8
NT = T // 128   # 16 token tiles
NQ = 4          # token quarters (512 tokens each)
NKT = C // 128  # 8 contraction tiles
EXP_SCALE = float(HD) ** -0.5
EXP = mybir.ActivationFunctionType.Exp
# attention i-chunks (start token-tile, tile count): taper the tail
CHUNKS = [(0, 4), (4, 4), (8, 4), (12, 2), (14, 1), (15, 1)]


def build_nc(split_waits=True):
    nc = bass.Bass()
    xb_d = nc.declare_dram_parameter("xb", [T, C], BF16, isOutput=False)
    wqk_d = nc.declare_dram_parameter("wqk", [C, 512], BF16, isOutput=False)
    bqk_d = nc.declare_dram_parameter("bqk", [512], F32, isOutput=False)
    wv_d = nc.declare_dram_parameter("wv", [C, 256], BF16, isOutput=False)
    wp_d = nc.declare_dram_parameter("wp", [256, C], BF16, isOutput=False)
    bp4_d = nc.declare_dram_parameter("bp4", [1, C], F32, isOutput=False)
    ident_d = nc.declare_dram_parameter("ident", [128, 128], BF16, isOutput=False)
    maskf_d = nc.declare_dram_parameter("maskf", [128, 896], BF16, isOutput=False)
    out_d = nc.declare_dram_parameter("out", [T, C], F32, isOutput=True)

    with tile.TileContext(nc) as tc:
        with (
            tc.tile_pool(name="const", bufs=1) as const,
            tc.tile_pool(name="wts", bufs=1) as wts,
            tc.tile_pool(name="xbs", bufs=1) as xbs,
            tc.tile_pool(name="xbp", bufs=3) as xbp,
            tc.tile_pool(name="xtp", bufs=2) as xtp,
            tc.tile_pool(name="qkt", bufs=1) as qkt,
            tc.tile_pool(name="vsb", bufs=1) as vsb,
            tc.tile_pool(name="ep", bufs=8) as ep,
            tc.tile_pool(name="rsp", bufs=3) as rsp,
            tc.tile_pool(name="ytp", bufs=1) as ytp,
            tc.tile_pool(name="outp", bufs=4) as outp,
            tc.tile_pool(name="psA", bufs=2, space="PSUM") as psA,
            tc.tile_pool(name="psE", bufs=2, space="PSUM") as psE,
            tc.tile_pool(name="psY", bufs=2, space="PSUM") as psY,
        ):
            # ---- x0 + ident first (gate the first transposes) ----
            xb_tiles = [None] * NT
            ident = const.tile([128, 128], BF16, name="ident")
            def load_x_tile(t):
                tl = xbs.tile([128, C], BF16, name="xb_t", tag=f"xbs{t}")
                nc.sync.dma_start(out=tl[:], in_=xb_d[t * 128:(t + 1) * 128, :])
                xb_tiles[t] = (tl, 0)
            def load_x_pair(t):
                tl = xbp.tile([128, 2 * C], BF16, name="xb_p", tag="xb_p")
                nc.sync.dma_start(
                    out=tl[:],
                    in_=xb_d[t * 128:(t + 2) * 128, :].rearrange(
                        "(a p) c -> p a c", p=128))
                xb_tiles[t] = (tl, 0)
                xb_tiles[t + 1] = (tl, 1)
            load_x_tile(0)
            nc.sync.dma_start(out=ident[:], in_=ident_d[:, :])
            for t in range(1, 4):
                load_x_tile(t)

            # ---- weights (ordered by first use) ----
            wqk_sb = []
            for h in range(2):
                w = wts.tile([128, 2048], BF16, name=f"wqk{h}", tag=f"wqk{h}")
                nc.sync.dma_start(
                    out=w[:],
                    in_=wqk_d[h * 512:(h + 1) * 512, :].rearrange(
                        "(a p) n -> p a n", p=128))
                wqk_sb.append(w)
            def wqk_ap(kt, p0, p1):
                return wqk_sb[kt // 4][:, (kt % 4) * 512 + p0:(kt % 4) * 512 + p1]

            bqk_sb = const.tile([128, 4], F32, name="bqk_sb")
            nc.sync.dma_start(out=bqk_sb[:], in_=bqk_d.rearrange("(t p) -> p t", p=128))

            wv_sb = []
            for h in range(2):
                w = wts.tile([128, 4 * 256], BF16, name=f"wv{h}", tag=f"wv{h}")
                nc.sync.dma_start(
                    out=w[:],
                    in_=wv_d[h * 512:(h + 1) * 512, :].rearrange(
                        "(a p) n -> p a n", p=128))
                wv_sb.append(w)
            def wv_ap(kt):
                return wv_sb[kt // 4][:, (kt % 4) * 256:(kt % 4 + 1) * 256]

            wp_sb = wts.tile([128, 2048], BF16, name="wp", tag="wp")
            nc.sync.dma_start(
                out=wp_sb[:],
                in_=wp_d[:, :].rearrange("(a p) n -> p a n", p=128))
            def wp_ap(kt, n0, n1):
                return wp_sb[:, kt * C + n0:kt * C + n1]

            bp_row = const.tile([1, C], F32, name="bp_row")
            nc.sync.dma_start(out=bp_row[:], in_=bp4_d[:, :])

            # remaining x tiles
            for t in range(4, NT, 2):
                load_x_pair(t)

            # ---- constants (ident/mask DMA'd: keeps Pool off the
            # startup critical path) ----
            maskf = const.tile([128, 896], BF16, name="maskf")
            nc.sync.dma_start(out=maskf[:], in_=maskf_d[:, :])
            # proj bias broadcast tile [128, C] via Pool SWDGE repeat-DMA
            bp_sb = const.tile([128, C], F32, name="bp_sb")
            nc.gpsimd.dma_start(
                out=bp_sb[:],
                in_=bp_row[0:1, :].unsqueeze(1).broadcast_to([1, 128, C]))

            # ---- persistent activations ----
            qkT = [qkt.tile([128, T], BF16, name=f"qkT{p}", tag=f"qkT{p}")
                   for p in range(4)]
            v_sb = [vsb.tile([128, 512], BF16, name=f"v{jt}", tag=f"v{jt}")
                    for jt in range(NT)]
            for jt in range(NT):
                # ones columns 64..127 of each head block: the y matmul then
                # replicates the softmax sums into out partitions 64..127
                nc.gpsimd.memset(
                    v_sb[jt][:].rearrange("p (s c) -> p s c", s=4)[:, :, 64:128],
                    1.0)
            yT = [[ytp.tile([128, 512], BF16, name=f"yT{ic}_{kt}",
                            tag=f"yT{ic}_{kt}") for kt in range(2)]
                  for ic in range(NQ)]

            # ---- attention for i-chunk [t0*128, (t0+nt)*128) as units ----
            def attn_chunk_units(t0, nt, split=False):
                w = nt * 128
                i0 = t0 * 128
                qq = t0 // 4
                units = []
                for s in range(4):
                    qrow = (s % 2) * 64
                    qtile = qkT[s // 2]
                    ktile = qkT[2 + s // 2]
                    n_jt = t0 + nt
                    state = {}

                    gsz = max(2, 1024 // w)

                    def pair_unit(g0, s=s, qrow=qrow, qtile=qtile,
                                  ktile=ktile, n_jt=n_jt, state=state,
                                  gsz=gsz):
                        if g0 == 0:
                            state["ps_y"] = psY.tile([128, 512], F32,
                                                     name="ps_y", tag="psY")
                            state["n_y"] = 0
                        ps_y = state["ps_y"]
                        pair = list(range(g0, min(g0 + gsz, n_jt)))
                        ps_a = psE.tile([128, 1024], F32, name="ps_a",
                                        tag="psE")
                        e = ep.tile([128, 1024], BF16, name="e_t", tag="e_t")
                        blocks = []
                        for h, jt in enumerate(pair):
                            o = max(0, (jt - t0) * 128)
                            nc.tensor.matmul(
                                ps_a[:, h * w + o:(h + 1) * w],
                                ktile[qrow:qrow + 64,
                                      jt * 128:(jt + 1) * 128],
                                qtile[qrow:qrow + 64, i0 + o:i0 + w],
                                start=True, stop=True,
                            )
                            blocks.append((jt, h, o))
                        if all(o == 0 for _, _, o in blocks):
                            nc.scalar.activation(
                                e[:, 0:len(pair) * w],
                                ps_a[:, 0:len(pair) * w],
                                EXP, scale=EXP_SCALE)
                        else:
                            for jt, h, o in blocks:
                                nc.scalar.activation(
                                    e[:, h * w + o:(h + 1) * w],
                                    ps_a[:, h * w + o:(h + 1) * w],
                                    EXP, scale=EXP_SCALE)
                        for jt, h, o in blocks:
                            if jt >= t0:
                                nc.vector.tensor_mul(
                                    e[:, h * w + o:(h + 1) * w],
                                    e[:, h * w + o:(h + 1) * w],
                                    maskf[:, 384:384 + w - o])
                            nc.tensor.matmul(
                                ps_y[:, o:w],
                                v_sb[jt][:, s * 128:s * 128 + 128],
                                e[:, h * w + o:(h + 1) * w],
                                start=(state["n_y"] == 0),
                                stop=(state["n_y"] == n_jt - 1),
                            )
                            state["n_y"] += 1

                    for g0 in range(0, n_jt, gsz):
                        units.append(lambda g0=g0, f=pair_unit: f(g0))

                    def norm_unit(s=s, qrow=qrow, state=state):
                        ps_y = state["ps_y"]
                        r_sb = rsp.tile([64, 512], BF16, name="r_sb",
                                        tag="r_sb")
                        with nc.allow_low_precision(
                                reason="softmax sum recip is well-conditioned"):
                            nc.vector.reciprocal(r_sb[:, 0:w],
                                                 ps_y[64:128, 0:w])
                        nc.vector.tensor_mul(
                            yT[qq][s // 2][qrow:qrow + 64,
                                           (t0 % 4) * 128:(t0 % 4) * 128 + w],
                            ps_y[0:64, 0:w], r_sb[:, 0:w])
                    units.append(norm_unit)

                # projection for this chunk's token tiles
                for mt in range(t0, t0 + nt):
                    def proj_unit(mt=mt, qq=qq):
                        mtl = mt % 4
                        o = outp.tile([128, C], F32, name="o_t", tag="o_t")
                        for nch in range(2):
                            sl = slice(nch * 512, (nch + 1) * 512)
                            ps = psA.tile([128, 512], F32, name="ps_o",
                                          tag="psA")
                            for kt in range(2):
                                nc.tensor.matmul(
                                    ps[:],
                                    yT[qq][kt][:, mtl * 128:(mtl + 1) * 128],
                                    wp_ap(kt, nch * 512, (nch + 1) * 512),
                                    start=(kt == 0), stop=(kt == 1),
                                )
                            nc.vector.tensor_add(o[:, sl], ps[:], bp_sb[:, sl])
                            nc.sync.dma_start(
                                out=out_d[mt * 128:(mt + 1) * 128, sl],
                                in_=o[:, sl])
                    units.append(proj_unit)
                if split:
                    return units[:-nt], units[-nt:]
                return units

            # ---- ABC units for one token-quarter ----
            def abc_units(Q):
                units = []
                xT_q = []

                def a_unit(kt, Q=Q, xT_q=xT_q):
                    xt = xtp.tile([128, 512], BF16, name="xT_t", tag=f"xT{kt}")
                    ps = psA.tile([128, 512], BF16, name="ps_tr", tag="psA")
                    for q in range(4):
                        tl, a = xb_tiles[4 * Q + q]
                        nc.tensor.transpose(
                            ps[:, q * 128:(q + 1) * 128],
                            tl[:, a * C + kt * 128:a * C + (kt + 1) * 128],
                            ident[:],
                        )
                    nc.vector.tensor_copy(xt[:], ps[:])
                    xT_q.append(xt)

                def b_unit(p, Q=Q, xT_q=xT_q):
                    ps = psA.tile([128, 512], F32, name="ps_qk", tag="psA")
                    for kt in range(NKT):
                        nc.tensor.matmul(
                            ps[:], wqk_ap(kt, p * 128, (p + 1) * 128),
                            xT_q[kt][:],
                            start=(kt == 0), stop=(kt == NKT - 1),
                        )
                    nc.scalar.activation(
                        qkT[p][:, Q * 512:(Q + 1) * 512], ps[:],
                        mybir.ActivationFunctionType.Identity,
                        bias=bqk_sb[:, p:p + 1])

                def c_unit(mt, Q=Q, xT_q=xT_q):
                    jt = 4 * Q + mt
                    ps = psA.tile([128, 256], F32, name="ps_v", tag="psA")
                    for kt in range(NKT):
                        nc.tensor.matmul(
                            ps[:], xT_q[kt][:, mt * 128:(mt + 1) * 128],
                            wv_ap(kt),
                            start=(kt == 0), stop=(kt == NKT - 1),
                        )
                    nc.scalar.copy(
                        v_sb[jt][:].rearrange("p (s c) -> p s c",
                                              s=4)[:, :, 0:64],
                        ps[:].rearrange("p (s c) -> p s c", s=4))

                for kt in range(NKT):
                    units.append(lambda kt=kt: a_unit(kt))
                for p in range(4):
                    units.append(lambda p=p: b_unit(p))
                for mt in range(4):
                    units.append(lambda mt=mt: c_unit(mt))
                return units

            def zip_emit(ua, ub):
                na, nb = len(ua), len(ub)
                ia = ib = 0
                while ia < na or ib < nb:
                    if ib >= nb or (ia < na and ia * nb <= ib * na):
                        ua[ia]()
                        ia += 1
                    else:
                        ub[ib]()
                        ib += 1

            # ---- main schedule: zip quarter ABC with prior chunk's attn;
            # chunk (8,4)'s projection is deferred into the tail as PE
            # filler for the ACT-bound final chunks ----
            for u in abc_units(0):
                u()
            au1, pu1 = attn_chunk_units(0, 4, split=True)
            zip_emit(abc_units(1), au1)
            au2, pu2 = attn_chunk_units(4, 4, split=True)
            zip_emit(abc_units(2), au2)
            au, pu = attn_chunk_units(8, 4, split=True)
            zip_emit(abc_units(3), au)
            tail = []
            for t0, nt in CHUNKS[3:]:
                tail.extend(attn_chunk_units(t0, nt))
            zip_emit(tail, pu1 + pu2 + pu)

    if split_waits:
        _split_matmul_waits(nc)
    return nc


def _split_matmul_waits(nc):
    """Walrus codegen in this pipeline allows only one sync wait per
    instruction for most ISA structs (S3_LW, PSEUDO_DMA_DIRECT2D, S3D3_TS,
    ...). Move extra waits onto inserted NoOps on the same engine (program
    order preserves semantics)."""
    n_split = 0
    for bb in nc.main_func.blocks:
        out = []
        for ins in bb.instructions:
            si = getattr(ins, "sync_info", None)
            if (si is not None and len(si.on_wait) >= 2
                    and type(ins).__name__ != "InstNoOp"):
                for w in si.on_wait[:-1]:
                    nop = mybir.InstNoOp(name=f"I-wsplit-{nc.next_id()}",
                                         ins=[], outs=[])
                    nop.engine = ins.engine
                    nop.sync_info = mybir.SyncInfo(on_wait=[w], on_update=[])
                    out.append(nop)
                    n_split += 1
                ins.sync_info = mybir.SyncInfo(
                    on_wait=[si.on_wait[-1]], on_update=si.on_update)
            out.append(ins)
        bb.instructions[:] = out
    return n_split


def shard_inputs(x, Wqkv, bqkv, Wproj, bproj):
    import ml_dtypes
    BF = ml_dtypes.bfloat16
    x = np.ascontiguousarray(np.asarray(x, np.float32))
    Wqkv = np.asarray(Wqkv, np.float32)
    bqkv = np.asarray(bqkv, np.float32)
    Wproj = np.asarray(Wproj, np.float32)
    bproj = np.asarray(bproj, np.float32)
    in_maps = []
    for c in range(N_CORES):
        b, hg = c // 4, c % 4
        wqk = np.ascontiguousarray(np.concatenate(
            [Wqkv[:, hg * 256:(hg + 1) * 256],
             Wqkv[:, C + hg * 256: C + (hg + 1) * 256]], axis=1).astype(BF))
        bqk = np.ascontiguousarray(np.concatenate(
            [bqkv[hg * 256:(hg + 1) * 256],
             bqkv[C + hg * 256: C + (hg + 1) * 256]]))
        wv = np.zeros((C, 256), np.float32)
        bv = np.zeros(256, np.float32)
        for s in range(4):
            h = 4 * hg + s
            wv[:, s * 64:s * 64 + 64] = Wqkv[:, 2 * C + h * 64: 2 * C + (h + 1) * 64]
            bv[s * 64:s * 64 + 64] = bqkv[2 * C + h * 64: 2 * C + (h + 1) * 64]
        wp = np.ascontiguousarray(Wproj[hg * 256:(hg + 1) * 256, :].astype(BF))
        # v-bias enters y additively (softmax rows sum to 1): fold exactly
        # into this core's projection bias partial.
        bp4 = np.ascontiguousarray(
            (bproj / 4 + bv @ Wproj[hg * 256:(hg + 1) * 256, :])[None, :])
        ident = np.ascontiguousarray(np.eye(128, dtype=np.float32).astype(BF))
        maskf = np.zeros((128, 896), np.float32)
        for j in range(128):
            maskf[j, j + 384:] = 1.0
        in_maps.append({"xb": np.ascontiguousarray(x[b].astype(BF)),
                        "wqk": wqk, "bqk": bqk,
                        "wv": np.ascontiguousarray(wv.astype(BF)),
                        "wp": wp, "bp4": bp4,
                        "ident": ident,
                        "maskf": np.ascontiguousarray(maskf.astype(BF))})
    return in_maps


_NC_CACHE = {}


def kernel(x, Wqkv, bqkv, Wproj, bproj):
    from concourse.bass_utils import run_bass_kernel_spmd

    if "nc" not in _NC_CACHE:
        _NC_CACHE["nc"] = build_nc()
    nc = _NC_CACHE["nc"]
    in_maps = shard_inputs(x, Wqkv, bqkv, Wproj, bproj)
    res = run_bass_kernel_spmd(nc, in_maps, list(range(N_CORES)))
    _NC_CACHE["last_exec_time_ns"] = res.exec_time_ns
    out = np.zeros((B, T, C), np.float32)
    for c in range(N_CORES):
        out[c // 4] += res.results[c]["out"]
    return out


# revision 36
# speedup vs baseline: 1.0497x; 1.0011x over previous
"""Causal self-attention (B=2, T=2048, C=1024, NH=16, HD=64) on 8 NeuronCores.

Sharding: core c -> (batch b = c//4, head-group hg = c%4 of 4 heads).
Each core computes qkv projection for its 4 heads from x[b], attention for
its 4 (b,h) units, and a partial output projection (row-parallel over the
head dim). Unshard = sum of the 4 partials per batch (bproj/4 folded in).

Per-core device algorithm (v3, bf16 data path):
  Host pre-casts x, Wqkv, Wproj shards to bf16 (rel-err budget ~4e-3 vs
  2e-2 gate; halves input DMA bytes).
  A. x[b] transposed on PE (128x128 bf16 tiles, 1 cyc/row) -> xT in SBUF
     (bf16 PSUM -> 2x-mode DVE evict).
  B. qkT [512, 2048] = wqk.T @ xT; DVE evict adds bias, keeps bf16.
  C. v_aug [2048, 260] = [x[b] | 1] @ wv_aug; ACT evict -> bf16 (per head
     64 v cols + a ones col: softmax sums as a matmul byproduct; v bias
     folded into the aug row).
  D. Attention in i-chunks (3x512 then 256/128/128 so the tail exposes
     little work): att^T[j,i] blocks via PE (K=64), j-tiles processed in
     pairs sharing a 2-bank PSUM so exp runs once per pair on ACT (scale
     1/8, no max subtraction -- logits are O(1) by construction) -> e bf16,
     causal mask = multiplicative bf16 mask on DVE (2x mode), y^T
     accumulation [65, w] (row 64 = softmax sums S).  Normalize: DVE
     recip(S) -> Pool partition_broadcast -> DVE mul -> yT bf16.
  E. out_partial = yT.T @ wp + bproj/4 per token-tile as soon as its chunk
     is normalized; f32 evict+bias on DVE, store via SP HWDGE DMA.
  Emission zips ABC units of quarter Q with attention units of the prior
  chunk so the Tile scheduler always has PE filler during exp stalls.
"""
import os
import sys

import numpy as np

for _p in ("/opt/trn_rl_repo",):
    if _p not in sys.path and os.path.isdir(_p):
        sys.path.insert(0, _p)

import concourse.bass as bass
import concourse.mybir as mybir
import concourse.tile as tile
from concourse.masks import make_identity

B, T, C, NH, HD = 2, 2048, 1024, 16, 64
F32 = mybir.dt.float32
BF16 = mybir.dt.bfloat16
N_CORES = 8
NT = T // 128   # 16 token tiles
NQ = 4          # token quarters (512 tokens each)
NKT = C // 128  # 8 contraction tiles
EXP_SCALE = float(HD) ** -0.5
EXP = mybir.ActivationFunctionType.Exp
# attention i-chunks (start token-tile, tile count): taper the tail
CHUNKS = [(0, 4), (4, 4), (8, 4), (12, 2), (14, 1), (15, 1)]


def build_nc(split_waits=True):
    nc = bass.Bass()
    xb_d = nc.declare_dram_parameter("xb", [T, C], BF16, isOutput=False)
    wqk_d = nc.declare_dram_parameter("wqk", [C, 512], BF16, isOutput=False)
    bqk_d = nc.declare_dram_parameter("bqk", [512], F32, isOutput=False)
    wv_d = nc.declare_dram_parameter("wv", [C, 256], BF16, isOutput=False)
    wp_d = nc.declare_dram_parameter("wp", [256, C], BF16, isOutput=False)
    bp4_d = nc.declare_dram_parameter("bp4", [1, C], F32, isOutput=False)
    ident_d = nc.declare_dram_parameter("ident", [128, 128], BF16, isOutput=False)
    maskf_d = nc.declare_dram_parameter("maskf", [128, 896], BF16, isOutput=False)
    out_d = nc.declare_dram_parameter("out", [T, C], F32, isOutput=True)

    with tile.TileContext(nc) as tc:
        with (
            tc.tile_pool(name="const", bufs=1) as const,
            tc.tile_pool(name="wts", bufs=1) as wts,
            tc.tile_pool(name="xbs", bufs=1) as xbs,
            tc.tile_pool(name="xbp", bufs=3) as xbp,
            tc.tile_pool(name="xtp", bufs=2) as xtp,
            tc.tile_pool(name="qkt", bufs=1) as qkt,
            tc.tile_pool(name="vsb", bufs=1) as vsb,
            tc.tile_pool(name="ep", bufs=8) as ep,
            tc.tile_pool(name="rsp", bufs=3) as rsp,
            tc.tile_pool(name="ytp", bufs=1) as ytp,
            tc.tile_pool(name="outp", bufs=6) as outp,
            tc.tile_pool(name="psA", bufs=2, space="PSUM") as psA,
            tc.tile_pool(name="psE", bufs=2, space="PSUM") as psE,
            tc.tile_pool(name="psY", bufs=2, space="PSUM") as psY,
        ):
            # ---- x0 + ident first (gate the first transposes) ----
            xb_tiles = [None] * NT
            ident = const.tile([128, 128], BF16, name="ident")
            def load_x_tile(t):
                tl = xbs.tile([128, C], BF16, name="xb_t", tag=f"xbs{t}")
                nc.sync.dma_start(out=tl[:], in_=xb_d[t * 128:(t + 1) * 128, :])
                xb_tiles[t] = (tl, 0)
            def load_x_pair(t):
                tl = xbp.tile([128, 2 * C], BF16, name="xb_p", tag="xb_p")
                nc.sync.dma_start(
                    out=tl[:],
                    in_=xb_d[t * 128:(t + 2) * 128, :].rearrange(
                        "(a p) c -> p a c", p=128))
                xb_tiles[t] = (tl, 0)
                xb_tiles[t + 1] = (tl, 1)
            load_x_tile(0)
            nc.sync.dma_start(out=ident[:], in_=ident_d[:, :])
            for t in range(1, 4):
                load_x_tile(t)

            # ---- weights (ordered by first use) ----
            wqk_sb = []
            for h in range(2):
                w = wts.tile([128, 2048], BF16, name=f"wqk{h}", tag=f"wqk{h}")
                nc.sync.dma_start(
                    out=w[:],
                    in_=wqk_d[h * 512:(h + 1) * 512, :].rearrange(
                        "(a p) n -> p a n", p=128))
                wqk_sb.append(w)
            def wqk_ap(kt, p0, p1):
                return wqk_sb[kt // 4][:, (kt % 4) * 512 + p0:(kt % 4) * 512 + p1]

            bqk_sb = const.tile([128, 4], F32, name="bqk_sb")
            nc.sync.dma_start(out=bqk_sb[:], in_=bqk_d.rearrange("(t p) -> p t", p=128))

            wv_sb = []
            for h in range(2):
                w = wts.tile([128, 4 * 256], BF16, name=f"wv{h}", tag=f"wv{h}")
                nc.sync.dma_start(
                    out=w[:],
                    in_=wv_d[h * 512:(h + 1) * 512, :].rearrange(
                        "(a p) n -> p a n", p=128))
                wv_sb.append(w)
            def wv_ap(kt):
                return wv_sb[kt // 4][:, (kt % 4) * 256:(kt % 4 + 1) * 256]

            wp_sb = wts.tile([128, 2048], BF16, name="wp", tag="wp")
            nc.sync.dma_start(
                out=wp_sb[:],
                in_=wp_d[:, :].rearrange("(a p) n -> p a n", p=128))
            def wp_ap(kt, n0, n1):
                return wp_sb[:, kt * C + n0:kt * C + n1]

            bp_row = const.tile([1, C], F32, name="bp_row")
            nc.sync.dma_start(out=bp_row[:], in_=bp4_d[:, :])

            # remaining x tiles
            for t in range(4, NT, 2):
                load_x_pair(t)

            # ---- constants (ident/mask DMA'd: keeps Pool off the
            # startup critical path) ----
            maskf = const.tile([128, 896], BF16, name="maskf")
            nc.sync.dma_start(out=maskf[:], in_=maskf_d[:, :])
            # proj bias broadcast tile [128, C] via Pool SWDGE repeat-DMA
            bp_sb = const.tile([128, C], F32, name="bp_sb")
            nc.gpsimd.dma_start(
                out=bp_sb[:],
                in_=bp_row[0:1, :].unsqueeze(1).broadcast_to([1, 128, C]))

            # ---- persistent activations ----
            qkT = [qkt.tile([128, T], BF16, name=f"qkT{p}", tag=f"qkT{p}")
                   for p in range(4)]
            v_sb = [vsb.tile([128, 512], BF16, name=f"v{jt}", tag=f"v{jt}")
                    for jt in range(NT)]
            for jt in range(NT):
                # ones columns 64..127 of each head block: the y matmul then
                # replicates the softmax sums into out partitions 64..127
                nc.gpsimd.memset(
                    v_sb[jt][:].rearrange("p (s c) -> p s c", s=4)[:, :, 64:128],
                    1.0)
            yT = [[ytp.tile([128, 512], BF16, name=f"yT{ic}_{kt}",
                            tag=f"yT{ic}_{kt}") for kt in range(2)]
                  for ic in range(NQ)]

            # ---- attention for i-chunk [t0*128, (t0+nt)*128) as units ----
            def attn_chunk_units(t0, nt, split=False):
                w = nt * 128
                i0 = t0 * 128
                qq = t0 // 4
                units = []
                for s in range(4):
                    qrow = (s % 2) * 64
                    qtile = qkT[s // 2]
                    ktile = qkT[2 + s // 2]
                    n_jt = t0 + nt
                    state = {}

                    gsz = max(2, 1024 // w)

                    def pair_unit(g0, s=s, qrow=qrow, qtile=qtile,
                                  ktile=ktile, n_jt=n_jt, state=state,
                                  gsz=gsz):
                        if g0 == 0:
                            state["ps_y"] = psY.tile([128, 512], F32,
                                                     name="ps_y", tag="psY")
                            state["n_y"] = 0
                        ps_y = state["ps_y"]
                        pair = list(range(g0, min(g0 + gsz, n_jt)))
                        ps_a = psE.tile([128, 1024], F32, name="ps_a",
                                        tag="psE")
                        e = ep.tile([128, 1024], BF16, name="e_t", tag="e_t")
                        blocks = []
                        for h, jt in enumerate(pair):
                            o = max(0, (jt - t0) * 128)
                            nc.tensor.matmul(
                                ps_a[:, h * w + o:(h + 1) * w],
                                ktile[qrow:qrow + 64,
                                      jt * 128:(jt + 1) * 128],
                                qtile[qrow:qrow + 64, i0 + o:i0 + w],
                                start=True, stop=True,
                            )
                            blocks.append((jt, h, o))
                        if all(o == 0 for _, _, o in blocks):
                            nc.scalar.activation(
                                e[:, 0:len(pair) * w],
                                ps_a[:, 0:len(pair) * w],
                                EXP, scale=EXP_SCALE)
                        else:
                            for jt, h, o in blocks:
                                nc.scalar.activation(
                                    e[:, h * w + o:(h + 1) * w],
                                    ps_a[:, h * w + o:(h + 1) * w],
                                    EXP, scale=EXP_SCALE)
                        for jt, h, o in blocks:
                            if jt >= t0:
                                nc.vector.tensor_mul(
                                    e[:, h * w + o:(h + 1) * w],
                                    e[:, h * w + o:(h + 1) * w],
                                    maskf[:, 384:384 + w - o])
                            nc.tensor.matmul(
                                ps_y[:, o:w],
                                v_sb[jt][:, s * 128:s * 128 + 128],
                                e[:, h * w + o:(h + 1) * w],
                                start=(state["n_y"] == 0),
                                stop=(state["n_y"] == n_jt - 1),
                            )
                            state["n_y"] += 1

                    for g0 in range(0, n_jt, gsz):
                        units.append(lambda g0=g0, f=pair_unit: f(g0))

                    def norm_unit(s=s, qrow=qrow, state=state):
                        ps_y = state["ps_y"]
                        r_sb = rsp.tile([64, 512], BF16, name="r_sb",
                                        tag="r_sb")
                        with nc.allow_low_precision(
                                reason="softmax sum recip is well-conditioned"):
                            nc.vector.reciprocal(r_sb[:, 0:w],
                                                 ps_y[64:128, 0:w])
                        nc.vector.tensor_mul(
                            yT[qq][s // 2][qrow:qrow + 64,
                                           (t0 % 4) * 128:(t0 % 4) * 128 + w],
                            ps_y[0:64, 0:w], r_sb[:, 0:w])
                    units.append(norm_unit)

                # projection for this chunk's token tiles
                for mt in range(t0, t0 + nt):
                    def proj_unit(mt=mt, qq=qq):
                        mtl = mt % 4
                        o = outp.tile([128, C], F32, name="o_t", tag="o_t")
                        for nch in range(2):
                            sl = slice(nch * 512, (nch + 1) * 512)
                            ps = psA.tile([128, 512], F32, name="ps_o",
                                          tag="psA")
                            for kt in range(2):
                                nc.tensor.matmul(
                                    ps[:],
                                    yT[qq][kt][:, mtl * 128:(mtl + 1) * 128],
                                    wp_ap(kt, nch * 512, (nch + 1) * 512),
                                    start=(kt == 0), stop=(kt == 1),
                                )
                            nc.vector.tensor_add(o[:, sl], ps[:], bp_sb[:, sl])
                            nc.sync.dma_start(
                                out=out_d[mt * 128:(mt + 1) * 128, sl],
                                in_=o[:, sl])
                    units.append(proj_unit)
                if split:
                    return units[:-nt], units[-nt:]
                return units

            # ---- ABC units for one token-quarter ----
            def abc_units(Q):
                units = []
                xT_q = []

                def a_unit(kt, Q=Q, xT_q=xT_q):
                    xt = xtp.tile([128, 512], BF16, name="xT_t", tag=f"xT{kt}")
                    ps = psA.tile([128, 512], BF16, name="ps_tr", tag="psA")
                    for q in range(4):
                        tl, a = xb_tiles[4 * Q + q]
                        nc.tensor.transpose(
                            ps[:, q * 128:(q + 1) * 128],
                            tl[:, a * C + kt * 128:a * C + (kt + 1) * 128],
                            ident[:],
                        )
                    nc.vector.tensor_copy(xt[:], ps[:])
                    xT_q.append(xt)

                def b_unit(p, Q=Q, xT_q=xT_q):
                    ps = psA.tile([128, 512], F32, name="ps_qk", tag="psA")
                    for kt in range(NKT):
                        nc.tensor.matmul(
                            ps[:], wqk_ap(kt, p * 128, (p + 1) * 128),
                            xT_q[kt][:],
                            start=(kt == 0), stop=(kt == NKT - 1),
                        )
                    nc.scalar.activation(
                        qkT[p][:, Q * 512:(Q + 1) * 512], ps[:],
                        mybir.ActivationFunctionType.Identity,
                        bias=bqk_sb[:, p:p + 1])

                def c_unit(mt, Q=Q, xT_q=xT_q):
                    jt = 4 * Q + mt
                    ps = psA.tile([128, 256], F32, name="ps_v", tag="psA")
                    for kt in range(NKT):
                        nc.tensor.matmul(
                            ps[:], xT_q[kt][:, mt * 128:(mt + 1) * 128],
                            wv_ap(kt),
                            start=(kt == 0), stop=(kt == NKT - 1),
                        )
                    nc.scalar.copy(
                        v_sb[jt][:].rearrange("p (s c) -> p s c",
                                              s=4)[:, :, 0:64],
                        ps[:].rearrange("p (s c) -> p s c", s=4))

                for kt in range(NKT):
                    units.append(lambda kt=kt: a_unit(kt))
                for p in range(4):
                    units.append(lambda p=p: b_unit(p))
                for mt in range(4):
                    units.append(lambda mt=mt: c_unit(mt))
                return units

            def zip_emit(ua, ub):
                na, nb = len(ua), len(ub)
                ia = ib = 0
                while ia < na or ib < nb:
                    if ib >= nb or (ia < na and ia * nb <= ib * na):
                        ua[ia]()
                        ia += 1
                    else:
                        ub[ib]()
                        ib += 1

            # ---- main schedule: zip quarter ABC with prior chunk's attn;
            # chunk (8,4)'s projection is deferred into the tail as PE
            # filler for the ACT-bound final chunks ----
            for u in abc_units(0):
                u()
            au1, pu1 = attn_chunk_units(0, 4, split=True)
            zip_emit(abc_units(1), au1)
            au2, pu2 = attn_chunk_units(4, 4, split=True)
            zip_emit(abc_units(2), au2)
            au, pu = attn_chunk_units(8, 4, split=True)
            zip_emit(abc_units(3), au)
            tail = []
            for t0, nt in CHUNKS[3:]:
                tail.extend(attn_chunk_units(t0, nt))
            zip_emit(tail, pu1 + pu2 + pu)

    if split_waits:
        _split_matmul_waits(nc)
    return nc


def _split_matmul_waits(nc):
    """Walrus codegen in this pipeline allows only one sync wait per
    instruction for most ISA structs (S3_LW, PSEUDO_DMA_DIRECT2D, S3D3_TS,
    ...). Move extra waits onto inserted NoOps on the same engine (program
    order preserves semantics)."""
    n_split = 0
    for bb in nc.main_func.blocks:
        out = []
        for ins in bb.instructions:
            si = getattr(ins, "sync_info", None)
            if (si is not None and len(si.on_wait) >= 2
                    and type(ins).__name__ != "InstNoOp"):
                for w in si.on_wait[:-1]:
                    nop = mybir.InstNoOp(name=f"I-wsplit-{nc.next_id()}",
                                         ins=[], outs=[])
                    nop.engine = ins.engine
                    nop.sync_info = mybir.SyncInfo(on_wait=[w], on_update=[])
                    out.append(nop)
                    n_split += 1
                ins.sync_info = mybir.SyncInfo(
                    on_wait=[si.on_wait[-1]], on_update=si.on_update)
            out.append(ins)
        bb.instructions[:] = out
    return n_split


def shard_inputs(x, Wqkv, bqkv, Wproj, bproj):
    import ml_dtypes
    BF = ml_dtypes.bfloat16
    x = np.ascontiguousarray(np.asarray(x, np.float32))
    Wqkv = np.asarray(Wqkv, np.float32)
    bqkv = np.asarray(bqkv, np.float32)
    Wproj = np.asarray(Wproj, np.float32)
    bproj = np.asarray(bproj, np.float32)
    in_maps = []
    for c in range(N_CORES):
        b, hg = c // 4, c % 4
        wqk = np.ascontiguousarray(np.concatenate(
            [Wqkv[:, hg * 256:(hg + 1) * 256],
             Wqkv[:, C + hg * 256: C + (hg + 1) * 256]], axis=1).astype(BF))
        bqk = np.ascontiguousarray(np.concatenate(
            [bqkv[hg * 256:(hg + 1) * 256],
             bqkv[C + hg * 256: C + (hg + 1) * 256]]))
        wv = np.zeros((C, 256), np.float32)
        bv = np.zeros(256, np.float32)
        for s in range(4):
            h = 4 * hg + s
            wv[:, s * 64:s * 64 + 64] = Wqkv[:, 2 * C + h * 64: 2 * C + (h + 1) * 64]
            bv[s * 64:s * 64 + 64] = bqkv[2 * C + h * 64: 2 * C + (h + 1) * 64]
        wp = np.ascontiguousarray(Wproj[hg * 256:(hg + 1) * 256, :].astype(BF))
        # v-bias enters y additively (softmax rows sum to 1): fold exactly
        # into this core's projection bias partial.
        bp4 = np.ascontiguousarray(
            (bproj / 4 + bv @ Wproj[hg * 256:(hg + 1) * 256, :])[None, :])
        ident = np.ascontiguousarray(np.eye(128, dtype=np.float32).astype(BF))
        maskf = np.zeros((128, 896), np.float32)
        for j in range(128):
            maskf[j, j + 384:] = 1.0
        in_maps.append({"xb": np.ascontiguousarray(x[b].astype(BF)),
                        "wqk": wqk, "bqk": bqk,
                        "wv": np.ascontiguousarray(wv.astype(BF)),
                        "wp": wp, "bp4": bp4,
                        "ident": ident,
                        "maskf": np.ascontiguousarray(maskf.astype(BF))})
    return in_maps


_NC_CACHE = {}


def kernel(x, Wqkv, bqkv, Wproj, bproj):
    from concourse.bass_utils import run_bass_kernel_spmd

    if "nc" not in _NC_CACHE:
        _NC_CACHE["nc"] = build_nc()
    nc = _NC_CACHE["nc"]
    in_maps = shard_inputs(x, Wqkv, bqkv, Wproj, bproj)
    res = run_bass_kernel_spmd(nc, in_maps, list(range(N_CORES)))
    _NC_CACHE["last_exec_time_ns"] = res.exec_time_ns
    out = np.zeros((B, T, C), np.float32)
    for c in range(N_CORES):
        out[c // 4] += res.results[c]["out"]
    return out
